# revision 9
# baseline (speedup 1.0000x reference)
"""Multi-head attention (B=2, S=2048, RES=1024, H=16) on 8 NeuronCores.

Sharding: batch*heads across cores. Core c handles batch c//4 and heads
4*(c%4) .. 4*(c%4)+3 (column-sharded QKV weights). No cross-core comm.

v2: ACT-saturation design. The softmax exp is ACT-engine-bound
(~1.29us per [128,1024] tile, 128 tiles = ~165us); everything else is
scheduled to keep the ACT pipeline fed from ~20us onward:
  - x arrives HOST-TRANSPOSED (xT [K, S]) so no PE transposes are needed
    and projections start as soon as the first xT chunk lands.
  - prologue projects only what the first iterations need: Q/K for heads
    0,1 plus V tiles 0..11; Q/K for heads 2,3, the rest of V, and the
    second half of Q0/1 are aux work interleaved into the attention loop.
  - one flat 128-iteration pipeline (8 head/s-half groups x 16 t-blocks):
    QK matmul pair -> exp on ACT -> PV pair lagged 2 iterations so the
    PV matmuls never reach the PE queue head before their `at` input is
    ready (avoids the LDW/MM stall observed in the baseline trace).
  - group order (h0,s0),(h1,s0),(h0,s1),(h1,s1),(h2,s0),(h3,s0),(h2,s1),
    (h3,s1): h2/h3 projections have until iteration 64 to complete as aux
    work, and the s0 output tiles finalize/DMA mid-loop.

Per-core kernel (S=2048, K=1024, C=256 = 4 heads x 64), bf16 matmuls
with fp32 PSUM accumulation:
  QT = (Wq_c)^T x^T  [C, S]      K on partitions (xT direct from DRAM)
  KT = (Wk_c)^T x^T  [C, S]
  V  = x Wv_c        [S, C] (+ ones col per head -> softmax sums ride
                             along in the PV matmul)
  per head: scoresT[t,s] = K_h^T Q_h -> exp(x/8) on ACT -> attnT (bf16)
            outT[d,s] (+ sums row) = V_aug^T attnT  (fp32 psum, 16 t-blocks)
            DMA xbar-transpose outT back to [s, d], rows * 1/sums, DMA out.

Q^T/K^T are stored per head with the 64 d-rows duplicated into partitions
64:128 so the two 512-wide QK matmuls of an iteration go to distinct PE
row groups.
"""

import sys

if "/opt/trn_rl_repo" not in sys.path:
    sys.path.insert(0, "/opt/trn_rl_repo")

import numpy as np

B = 2
S = 2048
RES = 1024
HEADS = 16
HD = 64  # head dim
N_CORES = 8
HPC = 4  # heads per core
C = HPC * HD  # 256 per-core projected width
K = RES  # contraction dim of projections
NKT = K // 128  # 8 k-chunks
NST = S // 128  # 16 s-tiles / t-blocks
SH = 1024  # s-half size for attention inner loop
VAUG = HD + 2  # 66: V cols + ones col + zero pad
PVLAG = 4  # PV consumes at from PVLAG iterations ago

_CACHE: dict = {}


def _build_nc():
    import concourse.mybir as mybir
    import concourse.tile as tile
    from concourse import bacc
    from concourse.masks import make_identity

    f32 = mybir.dt.float32
    bf16 = mybir.dt.bfloat16
    AF = mybir.ActivationFunctionType

    nc = bacc.Bacc(None)
    xt_in = nc.dram_tensor("xt", [K, S], bf16, kind="ExternalInput")
    wq_in = nc.dram_tensor("wq", [K, C], bf16, kind="ExternalInput")
    wk_in = nc.dram_tensor("wk", [K, C], bf16, kind="ExternalInput")
    wv_in = nc.dram_tensor("wv", [K, C], bf16, kind="ExternalInput")
    out_d = nc.dram_tensor("out", [S, C], f32, kind="ExternalOutput")

    # group order: (h, shi). h2/h3 delayed so their projections can be aux
    # work; s0 appears in the first half for both head pairs so its out
    # tiles finalize mid-loop.
    GROUPS = [(0, 0), (1, 0), (0, 1), (1, 1), (2, 0), (3, 0), (2, 1), (3, 1)]
    NITER = len(GROUPS) * NST  # 128

    with tile.TileContext(nc) as tc:
        with (
            tc.tile_pool(name="persist", bufs=1) as persist,
            tc.tile_pool(name="attn", bufs=2) as attn,
            tc.tile_pool(name="psum", bufs=1, space="PSUM") as ps,
        ):
            ident32 = persist.tile([128, 128], f32)
            make_identity(nc, ident32)
            ident = persist.tile([128, 128], bf16)
            nc.vector.tensor_copy(ident[:], ident32[:])
            ones4 = persist.tile([128, HPC], f32)
            nc.vector.memset(ones4[:], 1.0)
            zeros4 = persist.tile([128, HPC], f32)
            nc.vector.memset(zeros4[:], 0.0)

            # ---- input DMAs ----
            wq_t, wk_t, wv_t = [], [], []
            for kk in range(NKT):
                for lst, src, nm in ((wq_t, wq_in, "wq"), (wk_t, wk_in, "wk"),
                                     (wv_t, wv_in, "wv")):
                    t_ = persist.tile([128, C], bf16, name=f"{nm}_{kk}",
                                      tag=nm, bufs=NKT)
                    nc.gpsimd.dma_start(t_[:], src[kk * 128:(kk + 1) * 128, :])
                    lst.append(t_)

            # xt arrives in column halves: all chunks' s-cols [0:1024) first
            # (enough for the s0 projections and Vproj t<8), then [1024:2048)
            xt_sb = []
            for kk in range(NKT):
                t_ = persist.tile([128, S], bf16, name=f"xt_{kk}", tag="xt",
                                  bufs=NKT)
                xt_sb.append(t_)
            for half in range(2):
                cs = slice(half * SH, (half + 1) * SH)
                for kk in range(NKT):
                    nc.sync.dma_start(xt_sb[kk][:, cs],
                                      xt_in[kk * 128:(kk + 1) * 128, cs])

            # per-head Q^T/K^T with the head's 64 d-rows duplicated into
            # partitions 64:128 (distinct PE row groups for the QK pair)
            qt_tiles = []
            kt_tiles = []
            for h in range(HPC):
                qt = persist.tile([128, S], bf16, name=f"qt_{h}", tag="qt",
                                  bufs=HPC)
                kt = persist.tile([128, S], bf16, name=f"kt_{h}", tag="kt",
                                  bufs=HPC)
                qt_tiles.append(qt)
                kt_tiles.append(kt)

            v_aug = []
            for st in range(NST):
                va = persist.tile([128, HPC * VAUG], bf16, name=f"vaug_{st}",
                                  tag="vaug", bufs=NST)
                v_aug.append(va)

            out_tiles = []
            for sb in range(NST):
                ot = persist.tile([128, C], f32, name=f"out_{sb}", tag="ot",
                                  bufs=NST)
                out_tiles.append(ot)

            # ---- helpers ----
            def emit_warm(n):
                # tiny full-array matmuls into an sc-tagged psum tile to keep
                # the PE clock-gate warm; results never read.
                wm = ps.tile([128, SH], f32, name="warm", tag="sc", bufs=2)
                for w in range(n):
                    nc.tensor.matmul(
                        wm[:, (w % 8) * 64:(w % 8) * 64 + 64],
                        ident[:], ident[:, 0:64],
                        start=True, stop=True, skip_group_check=True,
                    )

            def finish_projqk(pp, dsts, half, sc):
                stg = attn.tile([128, 512], bf16, name=f"stg_{half}_{sc}",
                                tag="stg", bufs=2)
                nc.vector.tensor_copy(stg[:], pp[:])
                cols = slice(sc * 512, (sc + 1) * 512)
                for hh in range(2):
                    dst = dsts[2 * half + hh]
                    nc.vector.tensor_copy(dst[0:HD, cols],
                                          stg[hh * HD:(hh + 1) * HD, :])
                    nc.vector.tensor_copy(dst[HD:128, cols],
                                          stg[hh * HD:(hh + 1) * HD, :])

            def emit_projqk(w_t, dsts, half, sc):
                # one 512-col chunk of a Q/K projection for a head pair.
                # half=0 -> heads 0,1 ; half=1 -> heads 2,3
                pp = ps.tile([128, 512], f32, name=f"pp_{half}_{sc}", tag="pp",
                             bufs=2)
                for kk in range(NKT):
                    nc.tensor.matmul(
                        pp[:],
                        w_t[kk][:, half * 128:half * 128 + 128],
                        xt_sb[kk][:, sc * 512:(sc + 1) * 512],
                        start=(kk == 0),
                        stop=(kk == NKT - 1),
                    )
                finish_projqk(pp, dsts, half, sc)

            def emit_vproj(st):
                va3 = v_aug[st].rearrange("p (h d) -> p h d", h=HPC)
                vp = ps.tile([128, C], f32, name=f"vp_{st}", tag="pp", bufs=2)
                for kk in range(NKT):
                    nc.tensor.matmul(
                        vp[:],
                        xt_sb[kk][:, st * 128:(st + 1) * 128],
                        wv_t[kk][:],
                        start=(kk == 0),
                        stop=(kk == NKT - 1),
                    )
                nc.vector.tensor_copy(
                    va3[:, :, 0:HD], vp.rearrange("p (h d) -> p h d", h=HPC))
                nc.vector.tensor_copy(
                    va3[:, :, HD:HD + 1],
                    ones4.rearrange("p (h o) -> p h o", h=HPC))
                nc.vector.tensor_copy(
                    va3[:, :, HD + 1:HD + 2],
                    zeros4.rearrange("p (h o) -> p h o", h=HPC))

            # ---- prologue ----
            # K01 sc0 + Q01 sc0 accumulate kk-interleaved so each matmul runs
            # as soon as xt chunk kk lands (tracks the DMA); warm bursts keep
            # the PE clock-gate open across the DMA-wait gaps.
            emit_warm(16)
            ppk = ps.tile([128, 512], f32, name="ppk0", tag="pp", bufs=2)
            ppq = ps.tile([128, 512], f32, name="ppq0", tag="pp", bufs=2)
            for kk in range(NKT):
                nc.tensor.matmul(ppk[:], wk_t[kk][:, 0:128],
                                 xt_sb[kk][:, 0:512],
                                 start=(kk == 0), stop=(kk == NKT - 1))
                nc.tensor.matmul(ppq[:], wq_t[kk][:, 0:128],
                                 xt_sb[kk][:, 0:512],
                                 start=(kk == 0), stop=(kk == NKT - 1))
                emit_warm(8)
            finish_projqk(ppk, kt_tiles, 0, 0)
            finish_projqk(ppq, qt_tiles, 0, 0)
            emit_projqk(wq_t, qt_tiles, 0, 1)
            emit_vproj(0)
            emit_vproj(1)

            # ---- aux work queue (popped one per iteration) ----
            # HARD emission deadlines (PE queue order must respect producer
            # before consumer): v_aug[t] before slot t+2 (PV emission),
            # kt chunk c before iteration 4c (QK emission).
            aux = []
            aux.append(lambda: emit_vproj(2))
            aux.append(lambda: emit_vproj(3))
            aux.append(lambda: emit_projqk(wk_t, kt_tiles, 0, 1))
            aux.append(lambda: emit_vproj(4))
            aux.append(lambda: emit_vproj(5))
            aux.append(lambda: emit_vproj(6))
            aux.append(lambda: emit_projqk(wk_t, kt_tiles, 0, 2))
            aux.append(lambda: emit_vproj(7))
            aux.append(lambda: emit_vproj(8))
            aux.append(lambda: emit_vproj(9))
            aux.append(lambda: emit_projqk(wk_t, kt_tiles, 0, 3))
            aux.append(lambda: emit_vproj(10))
            aux.append(lambda: emit_vproj(11))
            aux.append(lambda: emit_vproj(12))
            aux.append(lambda: emit_vproj(13))
            aux.append(lambda: emit_vproj(14))
            aux.append(lambda: emit_vproj(15))
            aux.append(lambda: emit_projqk(wq_t, qt_tiles, 0, 2))
            aux.append(lambda: emit_projqk(wq_t, qt_tiles, 0, 3))
            # heads 2,3 (first used at iteration 64; s1 chunks at 96)
            aux.append(lambda: emit_projqk(wk_t, kt_tiles, 1, 0))
            aux.append(lambda: emit_projqk(wk_t, kt_tiles, 1, 1))
            aux.append(lambda: emit_projqk(wq_t, qt_tiles, 1, 0))
            aux.append(lambda: emit_projqk(wk_t, kt_tiles, 1, 2))
            aux.append(lambda: emit_projqk(wk_t, kt_tiles, 1, 3))
            aux.append(lambda: emit_projqk(wq_t, qt_tiles, 1, 1))
            aux.append(lambda: emit_projqk(wq_t, qt_tiles, 1, 2))
            aux.append(lambda: emit_projqk(wq_t, qt_tiles, 1, 3))
            AUX_EVERY = 1  # pop one aux task per iteration until drained

            # ---- attention pipeline ----
            pending = []  # (g, t, at_tile)
            outp_of = {}  # g -> psum tile
            tails = {0: [], 1: []}  # shi -> finished (h, oT) for normalize
            done_heads = {0: 0, 1: 0}

            def emit_pv(g, t, at):
                h, shi = GROUPS[g]
                if t == 0:
                    outp_of[g] = ps.tile([VAUG, SH], f32, name=f"outT_{g}",
                                         tag="outT", bufs=1)
                outp = outp_of[g]
                for scj in range(SH // 512):
                    nc.tensor.matmul(
                        outp[:, scj * 512:(scj + 1) * 512],
                        v_aug[t][:, h * VAUG:(h + 1) * VAUG],
                        at[:, scj * 512:(scj + 1) * 512],
                        start=(t == 0),
                        stop=(t == NST - 1),
                    )
                if t == NST - 1:
                    oT = attn.tile([80, SH], bf16, name=f"oT_{g}", tag="oT",
                                   bufs=4)
                    outp_done = outp_of.pop(g)
                    # copy in halves so the first starts while the second
                    # PV chunk is still streaming
                    nc.vector.tensor_copy(oT[0:VAUG, 0:512],
                                          outp_done[:, 0:512])
                    nc.vector.tensor_copy(oT[0:VAUG, 512:SH],
                                          outp_done[:, 512:SH])
                    emit_tail(g, oT)

            def emit_tail(g, oT):
                # DMA xbar transpose back to [s, d] (in halves, pipelined
                # with the normalize), then normalize rows by 1/sums
                # (col HD of the transposed block)
                h, shi = GROUPS[g]
                trb = attn.tile([128, (SH // 128) * 80], bf16,
                                name=f"trb_{g}", tag="trb", bufs=4)
                trb3 = trb.rearrange("p (j c) -> p j c", j=SH // 128)
                for jh in range(2):
                    jlo, jhi = jh * 4, jh * 4 + 4
                    nc.sync.dma_start_transpose(
                        trb3[:, jlo:jhi, :], oT[0:80, jlo * 128:jhi * 128])
                    for j in range(jlo, jhi):
                        sb = shi * (SH // 128) + j
                        rs = attn.tile([128, 1], f32, name=f"rs_{g}_{j}",
                                       tag="rs", bufs=8)
                        nc.vector.reciprocal(rs[:], trb3[:, j, HD:HD + 1])
                        nc.vector.tensor_scalar_mul(
                            out_tiles[sb][:, h * HD:(h + 1) * HD],
                            trb3[:, j, 0:HD], rs[:])
                        if done_heads[shi] == HPC - 1:
                            # last head of this s-half: DMA each out tile as
                            # soon as its normalize lands
                            nc.sync.dma_start(
                                out_d[sb * 128:(sb + 1) * 128, :],
                                out_tiles[sb][:])
                done_heads[shi] += 1

            def emit_qk(it):
                # returns (g, t, at) for the pending-PV queue
                g, t = divmod(it, NST)
                h, shi = GROUPS[g]
                qt = qt_tiles[h]
                kt = kt_tiles[h]
                s0 = shi * SH
                sc_ps = ps.tile([128, SH], f32, name=f"sc_{it}", tag="sc",
                                bufs=2)
                for scj in range(SH // 512):
                    dlo = scj * HD
                    dhi = dlo + HD
                    nc.tensor.matmul(
                        sc_ps[:, scj * 512:(scj + 1) * 512],
                        kt[dlo:dhi, t * 128:(t + 1) * 128],
                        qt[dlo:dhi, s0 + scj * 512:s0 + (scj + 1) * 512],
                        start=True, stop=True, skip_group_check=True,
                    )
                at = attn.tile([128, SH], bf16, name=f"at_{it}", tag="at",
                               bufs=PVLAG + 2)
                return g, t, at, sc_ps

            # iterations processed in PAIRS: both QK matmul pairs (64-row
            # tiling mode) back-to-back, then both exps, then aux + PV
            # (128-row mode) -- one PE tiling-mode drain per pair instead
            # of two per iteration.
            for p in range(NITER // 2):
                q0 = emit_qk(2 * p)
                q1 = emit_qk(2 * p + 1)
                for (g, t, at, sc_ps) in (q0, q1):
                    nc.scalar.activation(at[:], sc_ps[:], AF.Exp, scale=0.125)
                    pending.append((g, t, at))
                for _ in range(2 // AUX_EVERY if aux else 0):
                    if aux:
                        aux.pop(0)()
                while len(pending) > PVLAG:
                    emit_pv(*pending.pop(0))
            while pending:
                emit_pv(*pending.pop(0))

    nc.finalize()
    return nc


def _get_nc():
    if "nc" not in _CACHE:
        _CACHE["nc"] = _build_nc()
    return _CACHE["nc"]


def kernel(x, Wq, Wk, Wv):
    import ml_dtypes
    from concourse import bass_utils

    bf = ml_dtypes.bfloat16
    x = np.asarray(x, dtype=np.float32).astype(bf)
    Wq = np.asarray(Wq, dtype=np.float32).astype(bf)
    Wk = np.asarray(Wk, dtype=np.float32).astype(bf)
    Wv = np.asarray(Wv, dtype=np.float32).astype(bf)

    nc = _get_nc()
    in_maps = []
    for c in range(N_CORES):
        b = c // 4
        g = c % 4
        cols = slice(g * C, (g + 1) * C)
        in_maps.append(
            {
                "xt": np.ascontiguousarray(x[b].T),
                "wq": np.ascontiguousarray(Wq[:, cols]),
                "wk": np.ascontiguousarray(Wk[:, cols]),
                "wv": np.ascontiguousarray(Wv[:, cols]),
            }
        )

    res = bass_utils.run_bass_kernel_spmd(nc, in_maps, list(range(N_CORES)))
    _CACHE["last_results"] = res

    out = np.empty((B, S, RES), dtype=np.float32)
    for c in range(N_CORES):
        b = c // 4
        g = c % 4
        out[b, :, g * C : (g + 1) * C] = res.results[c]["out"]
    return out
